# revision 1
# baseline (speedup 1.0000x reference)
"""Sequence-parallel dense attention kernel for 8 Trainium2 NeuronCores.

Math (reference):
    h = x @ W1.T + b1                  [N, H]
    q/k/v = h @ W{q,k,v}.T + b{q,k,v}  [N, H]
    A = softmax(q @ k.T / sqrt(H))     [N, N]
    out = (h + A @ v) @ W2.T + b2      [N]

Algebraic restructuring:
  * out[n] = h[n]@w2 + (A_un[n,:]@z)/(A_un[n,:]@1) + (b_v@w2 + b2), where
    A_un = exp(scores) and z = v_nobias @ w2.  Softmax rows sum to one, so
    the v-bias contributes a constant and W2 (H->1) can be applied to V
    *before* attention — the whole [N,N]@[N,H] P@V matmul collapses into a
    [z | ones] reduction the PE does while streaming exp-scores once.
  * k = x @ (k_w @ lin1_w).T + (k_w@b1 + k_b): the k-projection is folded
    into one host-side weight so k.T is computed straight from x.T,
    concurrently with h.T — the all-gather input is ready ~25us in.
  * z = h @ (v_w.T @ w2): v is never materialized.

Sharding: rows of x across 8 cores (S = N/8 per core).  Scores are computed
transposed (ST[nk, nq] = k @ q.T) so the contraction of exp(ST) over nk is a
plain PE matmul (nk on partitions).  k.T (bf16) and z are all-gathered
(0.53MB/core, one packed AllGather).
"""

import numpy as np

N, D, H = 8192, 1024, 256
NC = 8
S = N // NC          # rows per core
NKC = N // 128       # 64 global nk chunks
SCALE = 0.0625       # 1/sqrt(256)

_cache = {}


def _build_program():
    import concourse.tile as tile
    from concourse import bacc, mybir
    from concourse.masks import make_identity

    f32 = mybir.dt.float32
    f32r = mybir.dt.float32r
    bf16 = mybir.dt.bfloat16
    Ident = mybir.ActivationFunctionType.Identity
    Exp = mybir.ActivationFunctionType.Exp
    Log = mybir.ActivationFunctionType.Ln

    nc = bacc.Bacc("TRN2", target_bir_lowering=False, debug=False, num_devices=NC)

    xT = nc.dram_tensor("xT", [D, S], f32r, kind="ExternalInput").ap()
    w1T = nc.dram_tensor("w1T", [D, H], f32r, kind="ExternalInput").ap()
    wk1T = nc.dram_tensor("wk1T", [D, H], f32r, kind="ExternalInput").ap()
    wqT = nc.dram_tensor("wqT", [H, H], f32r, kind="ExternalInput").ap()
    # packed small constants (per-partition columns):
    #   0-1 b1 | 2-3 bq | 4-5 bkk=k_w@b1+k_b | 6-7 w2 | 8 c0 | 9-10 wv2=v_w.T@w2
    #   11 zc0=wv2@b1
    cpk = nc.dram_tensor("cpk", [128, 16], f32, kind="ExternalInput").ap()
    # zw = lin1_w.T @ wv2 packed per d-chunk: col 2*dc = zw chunk, col 2*dc+1 = 0
    zwp = nc.dram_tensor("zwp", [128, 16], f32r, kind="ExternalInput").ap()
    out_d = nc.dram_tensor("out", [1, S], f32, kind="ExternalOutput").ap()

    cc_in = nc.dram_tensor("cc_in", [H + 1, S], bf16).ap()
    cc_out = nc.dram_tensor("cc_out", [(H + 1) * NC, S], bf16, addr_space="Shared").ap()

    with tile.TileContext(nc) as tc:
        with (
            tc.tile_pool(name="consts", bufs=1) as consts,
            tc.tile_pool(name="xpool", bufs=8) as xpool,
            tc.tile_pool(name="work", bufs=1) as work,
            tc.tile_pool(name="small", bufs=2) as small,
            tc.tile_pool(name="expp", bufs=11) as expp,
            tc.tile_pool(name="zrp", bufs=11) as zrp,
            tc.tile_pool(name="stp", bufs=3, space="PSUM") as stp,
            tc.tile_pool(name="redp", bufs=1, space="PSUM") as redp,
        ):
            # ---- interleaved chunk loads: PE can start after the first chunk ----
            w1sb = consts.tile([128, 8, H], f32r)
            wk1sb = consts.tile([128, 8, H], f32r)
            w1c = w1T.rearrange("(c p) h -> p c h", p=128)
            wk1c = wk1T.rearrange("(c p) h -> p c h", p=128)
            xts = []
            for dc in range(8):
                nc.sync.dma_start(out=wk1sb[:, dc, :], in_=wk1c[:, dc, :])
                xt = xpool.tile([128, S], f32r, tag="xt")
                nc.sync.dma_start(out=xt, in_=xT[dc * 128:(dc + 1) * 128, :])
                xts.append(xt)
            cpack = consts.tile([128, 16], f32)
            nc.sync.dma_start(out=cpack, in_=cpk)
            zwsb = consts.tile([128, 16], f32r)
            nc.sync.dma_start(out=zwsb, in_=zwp)
            # warm the ACT exp table set before any real activation needs it
            dumm = consts.tile([1, 1], f32)
            nc.vector.memset(dumm, 0.0)
            dumo = consts.tile([1, 1], f32)
            nc.scalar.activation(out=dumo, in_=dumm, func=Exp)

            # ---- ktloc = Wk1 @ x.T + bkk and z = zw @ x.T + zc0 (no h needed) ----
            ktloc = work.tile([128, 2, S], bf16)
            zrowsb = work.tile([1, S], bf16)
            for nt in range(2):
                for hc in range(2):
                    ps = stp.tile([128, 512], f32, tag="st", name="ps")
                    for dc in range(8):
                        nc.tensor.matmul(
                            ps,
                            lhsT=wk1sb[:, dc, hc * 128:(hc + 1) * 128],
                            rhs=xts[dc][:, nt * 512:(nt + 1) * 512],
                            start=(dc == 0),
                            stop=(dc == 7),
                        )
                    nc.scalar.activation(
                        out=ktloc[:, hc, nt * 512:(nt + 1) * 512], in_=ps,
                        func=Ident, bias=cpack[:, 4 + hc:4 + hc + 1],
                    )
                psz = stp.tile([2, 512], f32, tag="st", name="psz")
                for dc in range(8):
                    nc.tensor.matmul(
                        psz,
                        lhsT=zwsb[:, 2 * dc:2 * dc + 2],
                        rhs=xts[dc][:, nt * 512:(nt + 1) * 512],
                        start=(dc == 0),
                        stop=(dc == 7),
                    )
                nc.scalar.activation(
                    out=zrowsb[:, nt * 512:(nt + 1) * 512], in_=psz[0:1, :],
                    func=Ident, bias=cpack[0:1, 11:12],
                )

            # ---- ship k.T + z to collective input ----
            for hc in range(2):
                nc.sync.dma_start(
                    out=cc_in[hc * 128:(hc + 1) * 128, :], in_=ktloc[:, hc, :]
                )
            nc.sync.dma_start(out=cc_in[H:H + 1, :], in_=zrowsb)

            # ---- all-gather k.T + z (1MB + 4KB per rank) ----
            nc.gpsimd.collective_compute(
                "AllGather",
                mybir.AluOpType.bypass,
                replica_groups=[list(range(NC))],
                ins=[cc_in[:]],
                outs=[cc_out[:]],
            )

            # ---- hT, q.T and residual overlap the collective ----
            for dc in range(8):
                nc.sync.dma_start(out=w1sb[:, dc, :], in_=w1c[:, dc, :])
            hTsb = work.tile([128, 2, S], f32r)
            for hc in range(2):
                for nt in range(2):
                    ps = stp.tile([128, 512], f32, tag="st", name="ps")
                    for dc in range(8):
                        nc.tensor.matmul(
                            ps,
                            lhsT=w1sb[:, dc, hc * 128:(hc + 1) * 128],
                            rhs=xts[dc][:, nt * 512:(nt + 1) * 512],
                            start=(dc == 0),
                            stop=(dc == 7),
                        )
                    nc.scalar.activation(
                        out=hTsb[:, hc, nt * 512:(nt + 1) * 512], in_=ps,
                        func=Ident, bias=cpack[:, hc:hc + 1],
                    )
            wqsb = consts.tile([128, 2, H], f32r)
            nc.sync.dma_start(out=wqsb, in_=wqT.rearrange("(c p) h -> p c h", p=128))
            ident = consts.tile([128, 128], f32)
            make_identity(nc, ident)
            zcat = consts.tile([128, 128], bf16)
            identb = consts.tile([8, 8], bf16)
            nc.vector.tensor_copy(out=identb, in_=ident[0:8, 0:8])
            onesb = consts.tile([128, 64], f32)
            nc.vector.memset(onesb, 1.0)
            nc.vector.tensor_copy(out=zcat[:, 64:128], in_=onesb)
            onesrep = zcat[:, 64:128]

            qTsb = work.tile([128, 2, S], bf16)
            for hc in range(2):
                for nt in range(2):
                    ps = stp.tile([128, 512], f32, tag="st", name="ps")
                    for hic in range(2):
                        nc.tensor.matmul(
                            ps,
                            lhsT=wqsb[:, hic, hc * 128:(hc + 1) * 128],
                            rhs=hTsb[:, hic, nt * 512:(nt + 1) * 512],
                            start=(hic == 0),
                            stop=(hic == 1),
                        )
                    nc.scalar.activation(
                        out=qTsb[:, hc, nt * 512:(nt + 1) * 512], in_=ps,
                        func=Ident, bias=cpack[:, 2 + hc:2 + hc + 1],
                    )

            residsb = consts.tile([1, S], f32)
            for nt in range(2):
                psr = stp.tile([1, 512], f32, tag="st", name="psr")
                for hic in range(2):
                    nc.tensor.matmul(
                        psr,
                        lhsT=cpack[:, 6 + hic:7 + hic],
                        rhs=hTsb[:, hic, nt * 512:(nt + 1) * 512].bitcast(f32),
                        start=(hic == 0),
                        stop=(hic == 1),
                    )
                nc.vector.tensor_copy(out=residsb[:, nt * 512:(nt + 1) * 512], in_=psr)

            # ---- unpack gathered z into zcat columns (via PE transposes) ----
            cc3 = cc_out.rearrange("(r q) j -> r q j", q=H + 1)
            zrows = work.tile([8, S], bf16)
            nc.sync.dma_start(out=zrows, in_=cc3[:, H, :])
            zcatf = consts.tile([128, 64], f32)
            zcv = zcat[:, 0:64].rearrange("p (j f) -> p f j", f=8)
            zcvf = zcatf.rearrange("p (j f) -> p f j", f=8)
            for f in range(8):
                pzt = stp.tile([128, 8], bf16, tag="st", name="pzt")
                nc.tensor.transpose(
                    out=pzt, in_=zrows[:, f * 128:(f + 1) * 128],
                    identity=identb[:],
                )
                nc.vector.tensor_copy(out=zcv[:, f, :], in_=pzt)
                nc.vector.tensor_copy(out=zcvf[:, f, :], in_=pzt)

            # ---- load full k.T ----
            kt0 = work.tile([128, N], bf16)
            kt1 = work.tile([128, N], bf16)
            kts = (kt0, kt1)
            for r in range(NC):
                for hc in range(2):
                    nc.sync.dma_start(
                        out=kts[hc][:, r * S:(r + 1) * S],
                        in_=cc3[r, hc * 128:(hc + 1) * 128, :],
                    )

            # ---- main loop: ST = k @ q.T, exp, reduce against [z | 1] ----
            psred = [
                redp.tile([128, 512], f32, tag=f"red{nt}", name=f"psred{nt}")
                for nt in range(2)
            ]
            zcr = zcat.rearrange("p (two c) -> p c two", two=2)
            exps = {}

            def emit_reduce(g):
                e = exps.pop(g)
                # replicate [z_g | 1] to a full 128-wide stationary so the
                # reduce runs in the same 128x128 tile mode as the score
                # matmuls (no PE array mode-switch drains).  Output rows
                # alternate num/den copies; rows 0/1 are read.
                zr = zrp.tile([128, 128], bf16, tag="zrep", name="zr")
                zr2 = zr.rearrange("p (c two) -> p two c", two=2)
                nc.vector.tensor_scalar_mul(zr2[:, 0, :], onesrep, zcatf[:, g:g + 1])
                nc.vector.tensor_copy(out=zr2[:, 1, :], in_=onesrep)
                for nt in range(2):
                    nc.tensor.matmul(
                        psred[nt],
                        lhsT=zr,
                        rhs=e[:, nt * 512:(nt + 1) * 512],
                        start=(g == 0),
                        stop=(g == NKC - 1),
                    )

            GRP = 8
            for gb in range(0, NKC, GRP):
                for g in range(gb, gb + GRP):
                    st = stp.tile([128, 1024], f32, tag="st")
                    for hc in range(2):
                        for nt in range(2):
                            nc.tensor.matmul(
                                st[:, nt * 512:(nt + 1) * 512],
                                lhsT=kts[hc][:, g * 128:(g + 1) * 128],
                                rhs=qTsb[:, hc, nt * 512:(nt + 1) * 512],
                                start=(hc == 0),
                                stop=(hc == 1),
                            )
                    e = expp.tile([128, 1024], bf16, tag="expst")
                    nc.scalar.activation(out=e, in_=st, func=Exp, scale=SCALE)
                    exps[g] = e
                if gb > 0:
                    for g in range(gb - GRP, gb):
                        emit_reduce(g)
            for g in range(NKC - GRP, NKC):
                emit_reduce(g)

            # ---- epilogue: out = resid + num/den + c0 ----
            outsb = consts.tile([1, S], f32)
            for nt in range(2):
                ndsb = small.tile([2, 512], f32, tag="nd")
                nc.vector.tensor_copy(out=ndsb, in_=psred[nt][0:2, :])
                densb = small.tile([1, 512], f32, tag="den")
                nc.sync.dma_start(out=densb, in_=ndsb[1:2, :])
                lnd = small.tile([1, 512], f32, tag="lnd")
                nc.scalar.activation(out=lnd, in_=densb, func=Log)
                rden = small.tile([1, 512], f32, tag="rden")
                nc.scalar.activation(out=rden, in_=lnd, func=Exp, scale=-1.0)
                m = small.tile([1, 512], f32, tag="m")
                nc.vector.tensor_mul(m, ndsb[0:1, :], rden)
                m2 = small.tile([1, 512], f32, tag="m2")
                nc.vector.tensor_add(m2, m, residsb[:, nt * 512:(nt + 1) * 512])
                nc.scalar.activation(
                    out=outsb[:, nt * 512:(nt + 1) * 512], in_=m2,
                    func=Ident, bias=cpack[0:1, 8:9],
                )
            nc.sync.dma_start(out=out_d[:], in_=outsb)

    nc.compile()
    return nc


def _get_program():
    if "nc" not in _cache:
        _cache["nc"] = _build_program()
    return _cache["nc"]


def kernel(x, lin1_w, lin1_b, q_w, q_b, k_w, k_b, v_w, v_b, lin2_w, lin2_b):
    from concourse.bass_utils import run_bass_kernel_spmd

    x = np.asarray(x, dtype=np.float32)
    lin1_w = np.asarray(lin1_w, dtype=np.float32)
    lin1_b = np.asarray(lin1_b, dtype=np.float32)
    q_w = np.asarray(q_w, dtype=np.float32)
    q_b = np.asarray(q_b, dtype=np.float32)
    k_w = np.asarray(k_w, dtype=np.float32)
    k_b = np.asarray(k_b, dtype=np.float32)
    v_w = np.asarray(v_w, dtype=np.float32)
    v_b = np.asarray(v_b, dtype=np.float32)
    lin2_w = np.asarray(lin2_w, dtype=np.float32)
    lin2_b = np.asarray(lin2_b, dtype=np.float32)

    nc = _get_program()

    wk1 = (k_w.astype(np.float64) @ lin1_w.astype(np.float64)).astype(np.float32)
    bkk = (k_w.astype(np.float64) @ lin1_b.astype(np.float64)).astype(np.float32) + k_b
    w2 = lin2_w[0]                                  # [H]
    wv2 = (v_w.T.astype(np.float64) @ w2.astype(np.float64)).astype(np.float32)
    c0 = np.float32(v_b @ w2 + lin2_b[0])

    cpk = np.zeros((128, 16), dtype=np.float32)
    cpk[:, 0:2] = lin1_b.reshape(2, 128).T
    cpk[:, 2:4] = q_b.reshape(2, 128).T
    cpk[:, 4:6] = bkk.reshape(2, 128).T
    cpk[:, 6:8] = w2.reshape(2, 128).T
    cpk[0, 8] = c0
    cpk[:, 9:11] = wv2.reshape(2, 128).T
    zw = (lin1_w.T.astype(np.float64) @ wv2.astype(np.float64)).astype(np.float32)
    zc0 = np.float32(wv2.astype(np.float64) @ lin1_b.astype(np.float64))
    cpk[0, 11] = zc0
    zwp = np.zeros((128, 16), dtype=np.float32)
    zwp[:, 0::2] = zw.reshape(8, 128).T

    w1T = np.ascontiguousarray(lin1_w.T)            # [D, H]
    wk1T = np.ascontiguousarray(wk1.T)              # [D, H]
    wqT = np.ascontiguousarray(q_w.T)               # [H, H]

    in_maps = []
    for i in range(NC):
        in_maps.append({
            "xT": np.ascontiguousarray(x[i * S:(i + 1) * S, :].T),
            "w1T": w1T, "wk1T": wk1T, "wqT": wqT,
            "cpk": cpk, "zwp": zwp,
        })

    res = run_bass_kernel_spmd(nc, in_maps, core_ids=list(range(NC)))
    out = np.concatenate([res.results[i]["out"].reshape(S) for i in range(NC)])
    return out.astype(np.float32)



# revision 7
# speedup vs baseline: 1.0050x; 1.0050x over previous
"""Sequence-parallel dense attention kernel for 8 Trainium2 NeuronCores.

Math (reference):
    h = x @ W1.T + b1                  [N, H]
    q/k/v = h @ W{q,k,v}.T + b{q,k,v}  [N, H]
    A = softmax(q @ k.T / sqrt(H))     [N, N]
    out = (h + A @ v) @ W2.T + b2      [N]

Restructuring (v2, fp8):
  * All projections fold through lin1 on the host so q, k, z, resid all come
    straight from x:  q = x@(Wq@W1).T + bq', k = x@(Wk@W1).T + bk',
    z = v@w2 = x@zw + zc0, resid = h@w2 = x@wr + cr.  No h is ever computed.
  * Softmax rows sum to one, so out[n] = resid[n] + (A_un[n,:]@z)/(A_un[n,:]@1)
    with A_un = exp(scores); W2 is applied to V before attention.
  * q.T/k.T are scaled by 64 and stored fp8(e4m3); score matmuls use
    perf_mode=DoubleRow (contract 256 in one pass).  exp(score) in [0.28, 3.6]
    is stored fp8 and the num/den reduction is also a DoubleRow matmul
    (two 128-row nk chunks per pass) against a [z-replicated | ones]
    stationary (out rows 0..63 = num, 64..127 = den).
  * exp is split across engines: ScalarE ACT exp for ~60% of tiles, VectorE
    computes the rest as int8(a*score + b) bitcast to fp8 (Schraudolph: the
    int8 bit pattern IS an fp8 exp approximation); zr stationaries are
    replicated on GpSimd so the DVE only does exp work.
  * The k.T+z AllGather is fp8 (257 KB/rank) and split into two column-half
    collectives, each issued as soon as that half of the local k projection
    is done (~20us in, vs ~58us for the old bf16 monolithic gather); the q
    projection and second-half projections overlap the gathers.

Sharding: rows of x across 8 cores (S = N/8 per core); each core computes its
S x N score block transposed (nk on partitions) and reduces it on the PE.
"""

import numpy as np

N, D, H = 8192, 1024, 256
NC = 8
S = N // NC          # rows per core
NKC = N // 128       # 64 global nk chunks
NPAIR = NKC // 2     # 32 DoubleRow chunk pairs
SCQ = 64.0           # fp8 q/k prescale
SC_EXP = 0.0625 / (SCQ * SCQ)   # exp( st * SC_EXP )
# Schraudolph fp8e4m3: i8 = rint(st*A8 + B8); bitcast int8 -> fp8 approximates
# exp(st * SC_EXP).  B8 = 56 - c_opt, c_opt = 0.30 (numerically tuned).
A8 = 8.0 * 1.4426950408889634 * SC_EXP
B8 = 56.0 - 0.30
LAG = 3              # reduce of pair i is emitted with score of pair i+LAG
NZR = 6              # static zr stationaries in rotation

_cache = {}


def _build_program():
    import concourse.tile as tile
    from concourse import bacc, mybir
    from concourse.masks import make_identity

    f32 = mybir.dt.float32
    f32r = mybir.dt.float32r
    fp8 = mybir.dt.float8e4
    i8 = mybir.dt.int8
    Ident = mybir.ActivationFunctionType.Identity
    Exp = mybir.ActivationFunctionType.Exp
    DR = mybir.MatmulPerfMode.DoubleRow
    Mul = mybir.AluOpType.mult
    Add = mybir.AluOpType.add

    nc = bacc.Bacc("TRN2", target_bir_lowering=False, debug=False, num_devices=NC)

    xT = nc.dram_tensor("xT", [D, S], f32r, kind="ExternalInput").ap()
    wk1T = nc.dram_tensor("wk1T", [D, H], f32r, kind="ExternalInput").ap()
    wq1T = nc.dram_tensor("wq1T", [D, H], f32r, kind="ExternalInput").ap()
    # packed small constants (per-partition columns):
    #   0-1 bk1*64 | 2-3 bq1*64 | col4 p0: zc0 | col5 p1: cr
    cpk = nc.dram_tensor("cpk", [128, 16], f32, kind="ExternalInput").ap()
    # zwp: col 2*dc = zw chunk dc (z row), col 2*dc+1 = wr chunk dc (resid row)
    zwp = nc.dram_tensor("zwp", [128, 16], f32r, kind="ExternalInput").ap()
    out_d = nc.dram_tensor("out", [1, S], f32, kind="ExternalOutput").ap()

    # split collective: kt rows 0:256 ([hc*128+p, n]) + z row 256, fp8;
    # one tensor pair per column half of the local S
    cc_in = [nc.dram_tensor(f"cc_in{t}", [H + 1, 512], fp8).ap() for t in range(2)]
    cc_out = [
        nc.dram_tensor(f"cc_out{t}", [(H + 1) * NC, 512], fp8, addr_space="Shared").ap()
        for t in range(2)
    ]

    with tile.TileContext(nc) as tc:
        with (
            tc.tile_pool(name="consts", bufs=1) as consts,
            tc.tile_pool(name="xpool", bufs=8) as xpool,
            tc.tile_pool(name="work", bufs=1) as work,
            tc.tile_pool(name="expp", bufs=6) as expp,
            tc.tile_pool(name="stp", bufs=3, space="PSUM") as stp,
            tc.tile_pool(name="redp", bufs=1, space="PSUM") as redp,
        ):
            # ---- interleaved loads: x in column halves so nt=0 work (and
            # its collective) can start before nt=1 columns land ----
            wk1sb = consts.tile([128, 8, H], f32r)
            wq1sb = consts.tile([128, 8, H], f32r)
            wk1c = wk1T.rearrange("(c p) h -> p c h", p=128)
            wq1c = wq1T.rearrange("(c p) h -> p c h", p=128)
            xts = [xpool.tile([128, S], f32r, tag="xt", name="xt") for _ in range(8)]
            for dc in range(8):
                nc.sync.dma_start(out=wk1sb[:, dc, :], in_=wk1c[:, dc, :])
                nc.sync.dma_start(
                    out=xts[dc][:, 0:512], in_=xT[dc * 128:(dc + 1) * 128, 0:512]
                )
            cpack = consts.tile([128, 16], f32)
            nc.sync.dma_start(out=cpack, in_=cpk)
            zwsb = consts.tile([128, 16], f32r)
            nc.sync.dma_start(out=zwsb, in_=zwp)
            for dc in range(8):
                nc.sync.dma_start(
                    out=xts[dc][:, 512:1024],
                    in_=xT[dc * 128:(dc + 1) * 128, 512:1024],
                )
                nc.sync.dma_start(out=wq1sb[:, dc, :], in_=wq1c[:, dc, :])
            # warm the ACT exp table set before any real activation needs it
            dumm = consts.tile([1, 1], f32)
            nc.vector.memset(dumm, 0.0)
            dumo = consts.tile([1, 1], f32)
            nc.scalar.activation(out=dumo, in_=dumm, func=Exp)

            # ---- ktloc = Wk1 @ x.T (fp8, *64) and z/resid rows, by nt ----
            ktl8 = work.tile([128, 2, S], fp8)
            zrow8 = work.tile([1, S], fp8)
            residsb2 = consts.tile([2, S], f32)
            for nt in range(2):
                for hc in range(2):
                    ps = stp.tile([128, 512], f32, tag="st", name="ps")
                    for dc in range(8):
                        nc.tensor.matmul(
                            ps,
                            lhsT=wk1sb[:, dc, hc * 128:(hc + 1) * 128],
                            rhs=xts[dc][:, nt * 512:(nt + 1) * 512],
                            start=(dc == 0),
                            stop=(dc == 7),
                        )
                    nc.scalar.activation(
                        out=ktl8[:, hc, nt * 512:(nt + 1) * 512], in_=ps,
                        func=Ident, bias=cpack[:, hc:hc + 1], scale=SCQ,
                    )
                psz = stp.tile([2, 512], f32, tag="st", name="psz")
                for dc in range(8):
                    nc.tensor.matmul(
                        psz,
                        lhsT=zwsb[:, 2 * dc:2 * dc + 2],
                        rhs=xts[dc][:, nt * 512:(nt + 1) * 512],
                        start=(dc == 0),
                        stop=(dc == 7),
                    )
                # rows [z, resid] + per-partition bias [zc0, cr]; z then fp8
                nc.vector.tensor_scalar_add(
                    residsb2[0:2, nt * 512:(nt + 1) * 512], psz[0:2, :],
                    cpack[0:2, 4:5],
                )
                nc.vector.tensor_copy(
                    out=zrow8[:, nt * 512:(nt + 1) * 512],
                    in_=residsb2[0:1, nt * 512:(nt + 1) * 512],
                )
                # ship this half to its collective input and gather it
                for hc in range(2):
                    nc.sync.dma_start(
                        out=cc_in[nt][hc * 128:(hc + 1) * 128, :],
                        in_=ktl8[:, hc, nt * 512:(nt + 1) * 512],
                    )
                nc.sync.dma_start(
                    out=cc_in[nt][H:H + 1, :], in_=zrow8[:, nt * 512:(nt + 1) * 512]
                )
                nc.gpsimd.collective_compute(
                    "AllGather",
                    mybir.AluOpType.bypass,
                    replica_groups=[list(range(NC))],
                    ins=[cc_in[nt][:]],
                    outs=[cc_out[nt][:]],
                )

            # resid to partition 0 for the epilogue
            residsb = consts.tile([1, S], f32)
            nc.sync.dma_start(out=residsb, in_=residsb2[1:2, :])

            # ---- qT (overlaps the collectives) ----
            qt8 = work.tile([128, 2, S], fp8)
            for hc in range(2):
                for nt in range(2):
                    ps = stp.tile([128, 512], f32, tag="st", name="ps")
                    for dc in range(8):
                        nc.tensor.matmul(
                            ps,
                            lhsT=wq1sb[:, dc, hc * 128:(hc + 1) * 128],
                            rhs=xts[dc][:, nt * 512:(nt + 1) * 512],
                            start=(dc == 0),
                            stop=(dc == 7),
                        )
                    nc.scalar.activation(
                        out=qt8[:, hc, nt * 512:(nt + 1) * 512], in_=ps,
                        func=Ident, bias=cpack[:, 2 + hc:2 + hc + 1], scale=SCQ,
                    )

            # ---- constants for transposes / reduce stationaries ----
            ident = consts.tile([16, 16], f32)
            make_identity(nc, ident)
            ident8 = consts.tile([8, 8], fp8)
            nc.vector.tensor_copy(out=ident8, in_=ident[0:8, 0:8])
            ones64 = consts.tile([128, 64], fp8)
            nc.vector.memset(ones64, 1.0)
            # z columns, one per (chunk-in-rank, rank): zcv[:, c, r]
            zcatf = consts.tile([128, 64], f32)
            zcv = zcatf.rearrange("p (r f) -> p f r", f=8)
            # static zr stationaries: [128, 2, 128]; cols 0:64 get the two z
            # chunks replicated (gpsimd, per pair), cols 64:128 stay ones
            zrs = []
            for zi in range(NZR):
                zr = consts.tile([128, 2, 128], fp8, name=f"zr{zi}")
                nc.vector.memset(zr[:, 0, 64:128], 1.0)
                nc.vector.memset(zr[:, 1, 64:128], 1.0)
                zrs.append(zr)

            # ---- main loop ----
            kt8 = work.tile([128, 2, N], fp8)
            cc3 = [cc_out[t].rearrange("(r q) c -> r q c", q=H + 1) for t in range(2)]
            zrows8 = [work.tile([8, 512], fp8, name=f"zrows8_{t}") for t in range(2)]

            psred = [
                redp.tile([128, 512], f32, tag=f"red{t}", name=f"psred{t}")
                for t in range(2)
            ]

            def emit_load_half(t):
                # kt columns: rank r half t -> global cols r*S + t*512
                for r in range(NC):
                    for i in range(2):
                        nc.sync.dma_start(
                            out=kt8[:, i, r * S + t * 512: r * S + t * 512 + 512],
                            in_=cc3[t][r, i * 128:(i + 1) * 128, :],
                        )
                    nc.sync.dma_start(
                        out=zrows8[t][r:r + 1, :], in_=cc3[t][r, H:H + 1, :]
                    )
                for f in range(4):
                    # fp8 transpose mode requires output element step of 2
                    pzt = stp.tile([128, 8, 2], fp8, tag="st", name="pzt")
                    nc.tensor.transpose(
                        out=pzt[:, :, 0], in_=zrows8[t][:, f * 128:(f + 1) * 128],
                        identity=ident8,
                    )
                    nc.vector.tensor_copy(out=zcv[:, t * 4 + f, :], in_=pzt[:, :, 0])

            # pair schedule: 16 pairs per column half; each pair handles two
            # adjacent global 128-chunks of one rank.  (chunks, exp_engines)
            sched = []
            for t in range(2):
                for r in range(NC):
                    for cp in range(2):
                        chunks = []
                        for j in range(2):
                            c = t * 4 + 2 * cp + j      # chunk-in-rank 0..7
                            g = r * 8 + c               # global chunk
                            chunks.append((
                                kt8[:, :, g * 128:(g + 1) * 128],
                                zcv[:, c, r:r + 1],
                            ))
                        sched.append(chunks)

            e2s = {}

            def emit_score(pi):
                chunks = sched[pi]
                e2 = expp.tile([128, 2, S], fp8, tag="e2", name="e2")
                for j, (ktap, _zc) in enumerate(chunks):
                    st = stp.tile([128, 1024], f32, tag="st", name="st")
                    for nt in range(2):
                        nc.tensor.matmul(
                            st[:, nt * 512:(nt + 1) * 512],
                            lhsT=ktap,
                            rhs=qt8[:, :, nt * 512:(nt + 1) * 512],
                            perf_mode=DR,
                        )
                    # ~60/40 ACT/DVE split: j==1 tiles go to the DVE
                    # (Schraudolph) except every 5th pair
                    if j == 1 and pi % 5 != 4:
                        nc.vector.tensor_scalar(
                            out=e2[:, j, :].bitcast(i8), in0=st,
                            scalar1=A8, scalar2=B8, op0=Mul, op1=Add,
                        )
                    else:
                        nc.scalar.activation(
                            out=e2[:, j, :], in_=st, func=Exp, scale=SC_EXP
                        )
                e2s[pi] = e2

            def emit_reduce(pi):
                chunks = sched[pi]
                e2 = e2s.pop(pi)
                zr = zrs[pi % NZR]
                for j, (_ktap, zc) in enumerate(chunks):
                    nc.gpsimd.tensor_scalar_mul(zr[:, j, 0:64], ones64, zc)
                for nt in range(2):
                    nc.tensor.matmul(
                        psred[nt],
                        lhsT=zr,
                        rhs=e2[:, :, nt * 512:(nt + 1) * 512],
                        perf_mode=DR,
                        start=(pi == 0),
                        stop=(pi == NPAIR - 1),
                    )

            for pi in range(NPAIR + LAG):
                if pi == 0:
                    emit_load_half(0)
                if pi == 16:
                    emit_load_half(1)
                if pi < NPAIR:
                    emit_score(pi)
                if pi >= LAG:
                    emit_reduce(pi - LAG)

            # ---- epilogue: out = resid + num/den ----
            # psred rows: 0..63 num copies, 64..127 den copies
            dsb = consts.tile([65, S], f32)
            for nt in range(2):
                nc.vector.tensor_copy(
                    out=dsb[64:65, nt * 512:(nt + 1) * 512], in_=psred[nt][64:65, :]
                )
            dall = consts.tile([1, S], f32)
            nc.sync.dma_start(out=dall, in_=dsb[64:65, :])
            rden = consts.tile([1, S], f32)
            nc.vector.reciprocal(out=rden, in_=dall)
            m = consts.tile([1, S], f32)
            for nt in range(2):
                nc.vector.tensor_mul(
                    m[:, nt * 512:(nt + 1) * 512],
                    psred[nt][0:1, :],
                    rden[:, nt * 512:(nt + 1) * 512],
                )
            outsb = consts.tile([1, S], f32)
            nc.vector.tensor_add(outsb, m, residsb)
            nc.sync.dma_start(out=out_d[:], in_=outsb)

    nc.compile()
    return nc


def _get_program():
    if "nc" not in _cache:
        _cache["nc"] = _build_program()
    return _cache["nc"]


def kernel(x, lin1_w, lin1_b, q_w, q_b, k_w, k_b, v_w, v_b, lin2_w, lin2_b):
    from concourse.bass_utils import run_bass_kernel_spmd

    x = np.asarray(x, dtype=np.float32)
    lin1_w = np.asarray(lin1_w, dtype=np.float32)
    lin1_b = np.asarray(lin1_b, dtype=np.float32)
    q_w = np.asarray(q_w, dtype=np.float32)
    q_b = np.asarray(q_b, dtype=np.float32)
    k_w = np.asarray(k_w, dtype=np.float32)
    k_b = np.asarray(k_b, dtype=np.float32)
    v_w = np.asarray(v_w, dtype=np.float32)
    v_b = np.asarray(v_b, dtype=np.float32)
    lin2_w = np.asarray(lin2_w, dtype=np.float32)
    lin2_b = np.asarray(lin2_b, dtype=np.float32)

    nc = _get_program()

    wk1 = (k_w.astype(np.float64) @ lin1_w.astype(np.float64)).astype(np.float32)
    bk1 = (k_w.astype(np.float64) @ lin1_b.astype(np.float64)).astype(np.float32) + k_b
    wq1 = (q_w.astype(np.float64) @ lin1_w.astype(np.float64)).astype(np.float32)
    bq1 = (q_w.astype(np.float64) @ lin1_b.astype(np.float64)).astype(np.float32) + q_b
    w2 = lin2_w[0]                                  # [H]
    wv2 = (v_w.T.astype(np.float64) @ w2.astype(np.float64)).astype(np.float32)
    zw = (lin1_w.T.astype(np.float64) @ wv2.astype(np.float64)).astype(np.float32)
    zc0 = np.float32(wv2.astype(np.float64) @ lin1_b.astype(np.float64))
    wr = (lin1_w.T.astype(np.float64) @ w2.astype(np.float64)).astype(np.float32)
    cr = np.float32(lin1_b @ w2 + v_b @ w2 + lin2_b[0])

    cpk = np.zeros((128, 16), dtype=np.float32)
    cpk[:, 0:2] = (bk1 * SCQ).reshape(2, 128).T
    cpk[:, 2:4] = (bq1 * SCQ).reshape(2, 128).T
    cpk[0, 4] = zc0
    cpk[1, 4] = cr
    zwp = np.zeros((128, 16), dtype=np.float32)
    zwp[:, 0::2] = zw.reshape(8, 128).T
    zwp[:, 1::2] = wr.reshape(8, 128).T

    wk1T = np.ascontiguousarray(wk1.T)              # [D, H]
    wq1T = np.ascontiguousarray(wq1.T)              # [D, H]

    in_maps = []
    for i in range(NC):
        in_maps.append({
            "xT": np.ascontiguousarray(x[i * S:(i + 1) * S, :].T),
            "wk1T": wk1T, "wq1T": wq1T,
            "cpk": cpk, "zwp": zwp,
        })

    res = run_bass_kernel_spmd(nc, in_maps, core_ids=list(range(NC)))
    out = np.concatenate([res.results[i]["out"].reshape(S) for i in range(NC)])
    return out.astype(np.float32)


# revision 12
# speedup vs baseline: 1.0842x; 1.0788x over previous
"""Sequence-parallel dense attention kernel for 8 Trainium2 NeuronCores.

Math (reference):
    h = x @ W1.T + b1                  [N, H]
    q/k/v = h @ W{q,k,v}.T + b{q,k,v}  [N, H]
    A = softmax(q @ k.T / sqrt(H))     [N, N]
    out = (h + A @ v) @ W2.T + b2      [N]

Restructuring (v2, fp8):
  * All projections fold through lin1 on the host so q, k, z, resid all come
    straight from x:  q = x@(Wq@W1).T + bq', k = x@(Wk@W1).T + bk',
    z = v@w2 = x@zw + zc0, resid = h@w2 = x@wr + cr.  No h is ever computed.
  * Softmax rows sum to one, so out[n] = resid[n] + (A_un[n,:]@z)/(A_un[n,:]@1)
    with A_un = exp(scores); W2 is applied to V before attention.
  * q.T/k.T are scaled by 64 and stored fp8(e4m3); score matmuls use
    perf_mode=DoubleRow (contract 256 in one pass).  exp(score) in [0.28, 3.6]
    is stored fp8 and the num/den reduction is also a DoubleRow matmul
    (two 128-row nk chunks per pass) against a [z-replicated | ones]
    stationary (out rows 0..63 = num, 64..127 = den).
  * exp is split across engines: ScalarE ACT exp for ~60% of tiles, VectorE
    computes the rest as int8(a*score + b) bitcast to fp8 (Schraudolph: the
    int8 bit pattern IS an fp8 exp approximation); zr stationaries are
    replicated on GpSimd so the DVE only does exp work.
  * The k.T+z AllGather is fp8 (257 KB/rank) and split into two column-half
    collectives, each issued as soon as that half of the local k projection
    is done (~20us in, vs ~58us for the old bf16 monolithic gather); the q
    projection and second-half projections overlap the gathers.

Sharding: rows of x across 8 cores (S = N/8 per core); each core computes its
S x N score block transposed (nk on partitions) and reduces it on the PE.
"""

import numpy as np

N, D, H = 8192, 1024, 256
NC = 8
S = N // NC          # rows per core
NKC = N // 128       # 64 global nk chunks
NPAIR = NKC // 2     # 32 DoubleRow chunk pairs
SCQ = 64.0           # fp8 q/k prescale
SC_EXP = 0.0625 / (SCQ * SCQ)   # exp( st * SC_EXP )
# Schraudolph fp8e4m3: i8 = rint(st*A8 + B8); bitcast int8 -> fp8 approximates
# exp(st * SC_EXP).  B8 = 56 - c_opt, c_opt = 0.30 (numerically tuned).
A8 = 8.0 * 1.4426950408889634 * SC_EXP
B8 = 56.0 - 0.30
LAG = 3              # reduce of pair i is emitted with score of pair i+LAG
NZR = 6              # static zr stationaries in rotation

_cache = {}


def _build_program():
    import concourse.tile as tile
    from concourse import bacc, mybir
    from concourse.masks import make_identity

    f32 = mybir.dt.float32
    f32r = mybir.dt.float32r
    fp8 = mybir.dt.float8e4
    i8 = mybir.dt.int8
    Ident = mybir.ActivationFunctionType.Identity
    Exp = mybir.ActivationFunctionType.Exp
    DR = mybir.MatmulPerfMode.DoubleRow
    Mul = mybir.AluOpType.mult
    Add = mybir.AluOpType.add

    nc = bacc.Bacc("TRN2", target_bir_lowering=False, debug=False, num_devices=NC)

    xT = nc.dram_tensor("xT", [D, S], f32r, kind="ExternalInput").ap()
    wk1T = nc.dram_tensor("wk1T", [D, H], f32r, kind="ExternalInput").ap()
    wq1T = nc.dram_tensor("wq1T", [D, H], f32r, kind="ExternalInput").ap()
    # packed small constants (per-partition columns):
    #   0-1 bk1*64 | 2-3 bq1*64 | col4 p0: zc0 | col5 p1: cr
    cpk = nc.dram_tensor("cpk", [128, 16], f32, kind="ExternalInput").ap()
    # zwp: col 2*dc = zw chunk dc (z row), col 2*dc+1 = wr chunk dc (resid row)
    zwp = nc.dram_tensor("zwp", [128, 16], f32r, kind="ExternalInput").ap()
    out_d = nc.dram_tensor("out", [1, S], f32, kind="ExternalOutput").ap()

    # split collective: kt rows 0:256 ([hc*128+p, n]) + z row 256, fp8;
    # one tensor pair per column half of the local S
    cc_in = [nc.dram_tensor(f"cc_in{t}", [H + 1, 512], fp8).ap() for t in range(2)]
    cc_out = [
        nc.dram_tensor(f"cc_out{t}", [(H + 1) * NC, 512], fp8, addr_space="Shared").ap()
        for t in range(2)
    ]

    with tile.TileContext(nc) as tc:
        with (
            tc.tile_pool(name="consts", bufs=1) as consts,
            tc.tile_pool(name="xpool", bufs=8) as xpool,
            tc.tile_pool(name="work", bufs=1) as work,
            tc.tile_pool(name="expp", bufs=6) as expp,
            tc.tile_pool(name="stp", bufs=3, space="PSUM") as stp,
            tc.tile_pool(name="redp", bufs=1, space="PSUM") as redp,
        ):
            # ---- interleaved loads: x in column halves so nt=0 work (and
            # its collective) can start before nt=1 columns land ----
            wk1sb = consts.tile([128, 8, H], f32r)
            wq1sb = consts.tile([128, 8, H], f32r)
            wk1c = wk1T.rearrange("(c p) h -> p c h", p=128)
            wq1c = wq1T.rearrange("(c p) h -> p c h", p=128)
            xts = [xpool.tile([128, S], f32r, tag="xt", name="xt") for _ in range(8)]
            for dc in range(8):
                nc.sync.dma_start(out=wk1sb[:, dc, :], in_=wk1c[:, dc, :])
                nc.sync.dma_start(
                    out=xts[dc][:, 0:512], in_=xT[dc * 128:(dc + 1) * 128, 0:512]
                )
            cpack = consts.tile([128, 16], f32)
            nc.sync.dma_start(out=cpack, in_=cpk)
            zwsb = consts.tile([128, 16], f32r)
            nc.sync.dma_start(out=zwsb, in_=zwp)
            for dc in range(8):
                nc.sync.dma_start(
                    out=xts[dc][:, 512:1024],
                    in_=xT[dc * 128:(dc + 1) * 128, 512:1024],
                )
                nc.sync.dma_start(out=wq1sb[:, dc, :], in_=wq1c[:, dc, :])
            # warm the ACT exp table set before any real activation needs it
            dumm = consts.tile([1, 1], f32)
            nc.vector.memset(dumm, 0.0)
            dumo = consts.tile([1, 1], f32)
            nc.scalar.activation(out=dumo, in_=dumm, func=Exp)

            # ---- ktloc = Wk1 @ x.T (fp8, *64) and z/resid rows, by nt ----
            ktl8 = work.tile([128, 2, S], fp8)
            zrow8 = work.tile([1, S], fp8)
            residsb2 = consts.tile([2, S], f32)
            for nt in range(2):
                for hc in range(2):
                    ps = stp.tile([128, 512], f32, tag="st", name="ps")
                    for dc in range(8):
                        nc.tensor.matmul(
                            ps,
                            lhsT=wk1sb[:, dc, hc * 128:(hc + 1) * 128],
                            rhs=xts[dc][:, nt * 512:(nt + 1) * 512],
                            start=(dc == 0),
                            stop=(dc == 7),
                        )
                    nc.scalar.activation(
                        out=ktl8[:, hc, nt * 512:(nt + 1) * 512], in_=ps,
                        func=Ident, bias=cpack[:, hc:hc + 1], scale=SCQ,
                    )
                psz = stp.tile([2, 512], f32, tag="st", name="psz")
                for dc in range(8):
                    nc.tensor.matmul(
                        psz,
                        lhsT=zwsb[:, 2 * dc:2 * dc + 2],
                        rhs=xts[dc][:, nt * 512:(nt + 1) * 512],
                        start=(dc == 0),
                        stop=(dc == 7),
                    )
                # rows [z, resid] + per-partition bias [zc0, cr]; z then fp8
                nc.vector.tensor_scalar_add(
                    residsb2[0:2, nt * 512:(nt + 1) * 512], psz[0:2, :],
                    cpack[0:2, 4:5],
                )
                nc.vector.tensor_copy(
                    out=zrow8[:, nt * 512:(nt + 1) * 512],
                    in_=residsb2[0:1, nt * 512:(nt + 1) * 512],
                )
                # ship this half to its collective input and gather it
                for hc in range(2):
                    nc.sync.dma_start(
                        out=cc_in[nt][hc * 128:(hc + 1) * 128, :],
                        in_=ktl8[:, hc, nt * 512:(nt + 1) * 512],
                    )
                nc.sync.dma_start(
                    out=cc_in[nt][H:H + 1, :], in_=zrow8[:, nt * 512:(nt + 1) * 512]
                )
                nc.gpsimd.collective_compute(
                    "AllGather",
                    mybir.AluOpType.bypass,
                    replica_groups=[list(range(NC))],
                    ins=[cc_in[nt][:]],
                    outs=[cc_out[nt][:]],
                )

            # resid to partition 0 for the epilogue
            residsb = consts.tile([1, S], f32)
            nc.sync.dma_start(out=residsb, in_=residsb2[1:2, :])

            # ---- qT (overlaps the collectives) ----
            qt8 = work.tile([128, 2, S], fp8)
            for hc in range(2):
                for nt in range(2):
                    ps = stp.tile([128, 512], f32, tag="st", name="ps")
                    for dc in range(8):
                        nc.tensor.matmul(
                            ps,
                            lhsT=wq1sb[:, dc, hc * 128:(hc + 1) * 128],
                            rhs=xts[dc][:, nt * 512:(nt + 1) * 512],
                            start=(dc == 0),
                            stop=(dc == 7),
                        )
                    nc.scalar.activation(
                        out=qt8[:, hc, nt * 512:(nt + 1) * 512], in_=ps,
                        func=Ident, bias=cpack[:, 2 + hc:2 + hc + 1], scale=SCQ,
                    )

            # ---- constants for transposes / reduce stationaries ----
            ident = consts.tile([16, 16], f32)
            make_identity(nc, ident)
            ident8 = consts.tile([8, 8], fp8)
            nc.vector.tensor_copy(out=ident8, in_=ident[0:8, 0:8])
            # z columns in fp8, one per (rank, chunk-in-rank): zcat8[:, r*8+c]
            zcat8 = consts.tile([128, 64], fp8)
            # all 32 pair stationaries [128, pair, 2, 128]: cols 0:64 the two
            # z chunks replicated (built per half with broadcast DVE copies),
            # cols 64:128 ones.  Pair pi = t*16 + r*2 + cp handles global
            # chunks g = r*8 + t*4 + 2*cp + j.
            zrall = consts.tile([128, 32, 2, 128], fp8)
            for j in range(2):
                nc.vector.memset(zrall[:, :, j, 64:128], 1.0)
            zr6 = zrall.rearrange("p (t r cp) j c -> p t r cp j c", t=2, cp=2)
            zc3 = zcat8.rearrange("p (r f) -> p r f", f=8)

            # ---- main loop ----
            kt8 = work.tile([128, 2, N], fp8)
            cc3 = [cc_out[t].rearrange("(r q) c -> r q c", q=H + 1) for t in range(2)]
            zrows8 = [work.tile([8, 512], fp8, name=f"zrows8_{t}") for t in range(2)]

            psred = [
                redp.tile([128, 512], f32, tag=f"red{t}", name=f"psred{t}")
                for t in range(2)
            ]

            def emit_load_half(t):
                # kt columns: rank r half t -> global cols r*S + t*512
                for r in range(NC):
                    for i in range(2):
                        nc.sync.dma_start(
                            out=kt8[:, i, r * S + t * 512: r * S + t * 512 + 512],
                            in_=cc3[t][r, i * 128:(i + 1) * 128, :],
                        )
                    nc.sync.dma_start(
                        out=zrows8[t][r:r + 1, :], in_=cc3[t][r, H:H + 1, :]
                    )
                for f in range(4):
                    # fp8 transpose mode requires output element step of 2
                    pzt = stp.tile([128, 8, 2], fp8, tag="st", name="pzt")
                    nc.tensor.transpose(
                        out=pzt[:, :, 0], in_=zrows8[t][:, f * 128:(f + 1) * 128],
                        identity=ident8,
                    )
                    nc.vector.tensor_copy(
                        out=zc3[:, :, t * 4 + f], in_=pzt[:, :, 0]
                    )
                # replicate this half's z columns into the 16 pair
                # stationaries with two broadcast copies
                for cp in range(2):
                    src = zc3[:, :, t * 4 + 2 * cp: t * 4 + 2 * cp + 2]
                    nc.vector.tensor_copy(
                        out=zr6[:, t, :, cp, :, 0:64],
                        in_=src[:, :, :, None].broadcast_to([128, 8, 2, 64]),
                    )

            # pair schedule: 16 pairs per column half; each pair handles two
            # adjacent global 128-chunks of one rank
            sched = []
            for t in range(2):
                for r in range(NC):
                    for cp in range(2):
                        chunks = []
                        for j in range(2):
                            c = t * 4 + 2 * cp + j      # chunk-in-rank 0..7
                            g = r * 8 + c               # global chunk
                            chunks.append(kt8[:, :, g * 128:(g + 1) * 128])
                        sched.append(chunks)

            e2s = {}

            def emit_score(pi):
                chunks = sched[pi]
                e2 = expp.tile([128, 2, S], fp8, tag="e2", name="e2")
                for j, ktap in enumerate(chunks):
                    st = stp.tile([128, 1024], f32, tag="st", name="st")
                    for nt in range(2):
                        nc.tensor.matmul(
                            st[:, nt * 512:(nt + 1) * 512],
                            lhsT=ktap,
                            rhs=qt8[:, :, nt * 512:(nt + 1) * 512],
                            perf_mode=DR,
                        )
                    # ~60/40 ACT/DVE split: j==1 tiles go to the DVE
                    # (Schraudolph) except every 5th pair
                    if j == 1 and pi % 5 != 4:
                        nc.vector.tensor_scalar(
                            out=e2[:, j, :].bitcast(i8), in0=st,
                            scalar1=A8, scalar2=B8, op0=Mul, op1=Add,
                        )
                    else:
                        nc.scalar.activation(
                            out=e2[:, j, :], in_=st, func=Exp, scale=SC_EXP
                        )
                e2s[pi] = e2

            def emit_reduce(pi):
                e2 = e2s.pop(pi)
                zr = zrall[:, pi, :, :]
                for nt in range(2):
                    nc.tensor.matmul(
                        psred[nt],
                        lhsT=zr,
                        rhs=e2[:, :, nt * 512:(nt + 1) * 512],
                        perf_mode=DR,
                        start=(pi == 0),
                        stop=(pi == NPAIR - 1),
                    )

            for pi in range(NPAIR + LAG):
                if pi == 0:
                    emit_load_half(0)
                if pi == 16:
                    emit_load_half(1)
                if pi < NPAIR:
                    emit_score(pi)
                if pi >= LAG:
                    emit_reduce(pi - LAG)

            # ---- epilogue: out = resid + num/den ----
            # psred rows: 0..63 num copies, 64..127 den copies
            dsb = consts.tile([65, S], f32)
            for nt in range(2):
                nc.vector.tensor_copy(
                    out=dsb[64:65, nt * 512:(nt + 1) * 512], in_=psred[nt][64:65, :]
                )
            dall = consts.tile([1, S], f32)
            nc.sync.dma_start(out=dall, in_=dsb[64:65, :])
            rden = consts.tile([1, S], f32)
            nc.vector.reciprocal(out=rden, in_=dall)
            m = consts.tile([1, S], f32)
            for nt in range(2):
                nc.vector.tensor_mul(
                    m[:, nt * 512:(nt + 1) * 512],
                    psred[nt][0:1, :],
                    rden[:, nt * 512:(nt + 1) * 512],
                )
            outsb = consts.tile([1, S], f32)
            nc.vector.tensor_add(outsb, m, residsb)
            nc.sync.dma_start(out=out_d[:], in_=outsb)

    nc.compile()
    return nc


def _get_program():
    if "nc" not in _cache:
        _cache["nc"] = _build_program()
    return _cache["nc"]


def kernel(x, lin1_w, lin1_b, q_w, q_b, k_w, k_b, v_w, v_b, lin2_w, lin2_b):
    from concourse.bass_utils import run_bass_kernel_spmd

    x = np.asarray(x, dtype=np.float32)
    lin1_w = np.asarray(lin1_w, dtype=np.float32)
    lin1_b = np.asarray(lin1_b, dtype=np.float32)
    q_w = np.asarray(q_w, dtype=np.float32)
    q_b = np.asarray(q_b, dtype=np.float32)
    k_w = np.asarray(k_w, dtype=np.float32)
    k_b = np.asarray(k_b, dtype=np.float32)
    v_w = np.asarray(v_w, dtype=np.float32)
    v_b = np.asarray(v_b, dtype=np.float32)
    lin2_w = np.asarray(lin2_w, dtype=np.float32)
    lin2_b = np.asarray(lin2_b, dtype=np.float32)

    nc = _get_program()

    wk1 = (k_w.astype(np.float64) @ lin1_w.astype(np.float64)).astype(np.float32)
    bk1 = (k_w.astype(np.float64) @ lin1_b.astype(np.float64)).astype(np.float32) + k_b
    wq1 = (q_w.astype(np.float64) @ lin1_w.astype(np.float64)).astype(np.float32)
    bq1 = (q_w.astype(np.float64) @ lin1_b.astype(np.float64)).astype(np.float32) + q_b
    w2 = lin2_w[0]                                  # [H]
    wv2 = (v_w.T.astype(np.float64) @ w2.astype(np.float64)).astype(np.float32)
    zw = (lin1_w.T.astype(np.float64) @ wv2.astype(np.float64)).astype(np.float32)
    zc0 = np.float32(wv2.astype(np.float64) @ lin1_b.astype(np.float64))
    wr = (lin1_w.T.astype(np.float64) @ w2.astype(np.float64)).astype(np.float32)
    cr = np.float32(lin1_b @ w2 + v_b @ w2 + lin2_b[0])

    cpk = np.zeros((128, 16), dtype=np.float32)
    cpk[:, 0:2] = (bk1 * SCQ).reshape(2, 128).T
    cpk[:, 2:4] = (bq1 * SCQ).reshape(2, 128).T
    cpk[0, 4] = zc0
    cpk[1, 4] = cr
    zwp = np.zeros((128, 16), dtype=np.float32)
    zwp[:, 0::2] = zw.reshape(8, 128).T
    zwp[:, 1::2] = wr.reshape(8, 128).T

    wk1T = np.ascontiguousarray(wk1.T)              # [D, H]
    wq1T = np.ascontiguousarray(wq1.T)              # [D, H]

    in_maps = []
    for i in range(NC):
        in_maps.append({
            "xT": np.ascontiguousarray(x[i * S:(i + 1) * S, :].T),
            "wk1T": wk1T, "wq1T": wq1T,
            "cpk": cpk, "zwp": zwp,
        })

    res = run_bass_kernel_spmd(nc, in_maps, core_ids=list(range(NC)))
    out = np.concatenate([res.results[i]["out"].reshape(S) for i in range(NC)])
    return out.astype(np.float32)


# revision 18
# speedup vs baseline: 1.2952x; 1.1946x over previous
"""Sequence-parallel dense attention kernel for 8 Trainium2 NeuronCores.

Math (reference):
    h = x @ W1.T + b1                  [N, H]
    q/k/v = h @ W{q,k,v}.T + b{q,k,v}  [N, H]
    A = softmax(q @ k.T / sqrt(H))     [N, N]
    out = (h + A @ v) @ W2.T + b2      [N]

Restructuring (v2, fp8):
  * All projections fold through lin1 on the host so q, k, z, resid all come
    straight from x:  q = x@(Wq@W1).T + bq', k = x@(Wk@W1).T + bk',
    z = v@w2 = x@zw + zc0, resid = h@w2 = x@wr + cr.  No h is ever computed.
  * Softmax rows sum to one, so out[n] = resid[n] + (A_un[n,:]@z)/(A_un[n,:]@1)
    with A_un = exp(scores); W2 is applied to V before attention.
  * q.T/k.T are scaled by 64 and stored fp8(e4m3); score matmuls use
    perf_mode=DoubleRow (contract 256 in one pass).  exp(score) in [0.28, 3.6]
    is stored fp8 and the num/den reduction is also a DoubleRow matmul
    (two 128-row nk chunks per pass) against a [z-replicated | ones]
    stationary (out rows 0..63 = num, 64..127 = den).
  * exp is split across engines: ScalarE ACT exp for ~60% of tiles, VectorE
    computes the rest as int8(a*score + b) bitcast to fp8 (Schraudolph: the
    int8 bit pattern IS an fp8 exp approximation); zr stationaries are
    replicated on GpSimd so the DVE only does exp work.
  * The k.T+z AllGather is fp8 (257 KB/rank) and split into two column-half
    collectives, each issued as soon as that half of the local k projection
    is done (~20us in, vs ~58us for the old bf16 monolithic gather); the q
    projection and second-half projections overlap the gathers.

Sharding: rows of x across 8 cores (S = N/8 per core); each core computes its
S x N score block transposed (nk on partitions) and reduces it on the PE.
"""

import numpy as np

N, D, H = 8192, 1024, 256
NC = 8
S = N // NC          # rows per core
NKC = N // 128       # 64 global nk chunks
NPAIR = NKC // 2     # 32 DoubleRow chunk pairs
SCQ = 64.0           # fp8 q/k prescale
SC_EXP = 0.0625 / (SCQ * SCQ)   # exp( st * SC_EXP )
# Schraudolph fp8e4m3: i8 = rint(st*A8 + B8); bitcast int8 -> fp8 approximates
# exp(st * SC_EXP).  B8 = 56 - c_opt, c_opt = 0.30 (numerically tuned).
A8 = 8.0 * 1.4426950408889634 * SC_EXP
B8 = 56.0 - 0.30
LAG = 3              # reduce of pair i is emitted with score of pair i+LAG
NZR = 6              # static zr stationaries in rotation

_cache = {}


def _build_program():
    import concourse.tile as tile
    from concourse import bacc, mybir
    from concourse.masks import make_identity

    f32 = mybir.dt.float32
    f32r = mybir.dt.float32r
    fp8 = mybir.dt.float8e4
    i8 = mybir.dt.int8
    Ident = mybir.ActivationFunctionType.Identity
    Exp = mybir.ActivationFunctionType.Exp
    Log = mybir.ActivationFunctionType.Ln
    DR = mybir.MatmulPerfMode.DoubleRow
    Mul = mybir.AluOpType.mult
    Add = mybir.AluOpType.add

    nc = bacc.Bacc("TRN2", target_bir_lowering=False, debug=False, num_devices=NC)

    xT = nc.dram_tensor("xT", [D, S], f32r, kind="ExternalInput").ap()
    wk1T = nc.dram_tensor("wk1T", [D, H], f32r, kind="ExternalInput").ap()
    wq1T = nc.dram_tensor("wq1T", [D, H], f32r, kind="ExternalInput").ap()
    # packed small constants (per-partition columns):
    #   0-1 bk1*64 | 2-3 bq1*64 | col4 p0: zc0 | col5 p1: cr
    cpk = nc.dram_tensor("cpk", [128, 16], f32, kind="ExternalInput").ap()
    # zwp: col 2*dc = zw chunk dc (z row), col 2*dc+1 = wr chunk dc (resid row)
    zwp = nc.dram_tensor("zwp", [128, 16], f32r, kind="ExternalInput").ap()
    out_d = nc.dram_tensor("out", [1, S], f32, kind="ExternalOutput").ap()

    # split collective: kt rows 0:256 ([hc*128+p, n]) + z row 256, fp8;
    # one tensor pair per column half of the local S
    cc_in = [nc.dram_tensor(f"cc_in{t}", [H + 1, 512], fp8).ap() for t in range(2)]
    cc_out = [
        nc.dram_tensor(f"cc_out{t}", [(H + 1) * NC, 512], fp8, addr_space="Shared").ap()
        for t in range(2)
    ]

    with tile.TileContext(nc) as tc:
        with (
            tc.tile_pool(name="consts", bufs=1) as consts,
            tc.tile_pool(name="xpool", bufs=8) as xpool,
            tc.tile_pool(name="work", bufs=1) as work,
            tc.tile_pool(name="expp", bufs=6) as expp,
            tc.tile_pool(name="stp", bufs=3, space="PSUM") as stp,
            tc.tile_pool(name="redp", bufs=1, space="PSUM") as redp,
        ):
            # ---- interleaved loads: x in column halves so nt=0 work (and
            # its collective) can start before nt=1 columns land ----
            wk1sb = consts.tile([128, 8, H], f32r)
            wq1sb = consts.tile([128, 8, H], f32r)
            wk1c = wk1T.rearrange("(c p) h -> p c h", p=128)
            wq1c = wq1T.rearrange("(c p) h -> p c h", p=128)
            xts = [xpool.tile([128, S], f32r, tag="xt", name="xt") for _ in range(8)]
            nc.sync.dma_start(out=wk1sb, in_=wk1c)
            for dc in range(8):
                nc.sync.dma_start(
                    out=xts[dc][:, 0:512], in_=xT[dc * 128:(dc + 1) * 128, 0:512]
                )
            cpack = consts.tile([128, 16], f32)
            nc.sync.dma_start(out=cpack, in_=cpk)
            zwsb = consts.tile([128, 16], f32r)
            nc.sync.dma_start(out=zwsb, in_=zwp)
            for dc in range(8):
                nc.sync.dma_start(
                    out=xts[dc][:, 512:1024],
                    in_=xT[dc * 128:(dc + 1) * 128, 512:1024],
                )
            nc.sync.dma_start(out=wq1sb, in_=wq1c)
            # warm the ACT table set; Ln pulls in natural_log_exp_and_others,
            # which also contains exp, so the main loop and the epilogue
            # (ln + exp division) share one table set with no reloads
            dumm = consts.tile([1, 1], f32)
            nc.vector.memset(dumm, 1.0)
            dumo = consts.tile([1, 1], f32)
            nc.scalar.activation(out=dumo, in_=dumm, func=Log)

            # ---- ktloc = Wk1 @ x.T (fp8, *64) and z/resid rows, by nt ----
            ktl8 = work.tile([128, 2, S], fp8)
            zrow8 = work.tile([1, S], fp8)
            residsb2 = consts.tile([2, S], f32)
            for nt in range(2):
                for hc in range(2):
                    ps = stp.tile([128, 512], f32, tag="st", name="ps")
                    for dc in range(8):
                        nc.tensor.matmul(
                            ps,
                            lhsT=wk1sb[:, dc, hc * 128:(hc + 1) * 128],
                            rhs=xts[dc][:, nt * 512:(nt + 1) * 512],
                            start=(dc == 0),
                            stop=(dc == 7),
                        )
                    nc.scalar.activation(
                        out=ktl8[:, hc, nt * 512:(nt + 1) * 512], in_=ps,
                        func=Ident, bias=cpack[:, hc:hc + 1], scale=SCQ,
                    )
                psz = stp.tile([2, 512], f32, tag="st", name="psz")
                for dc in range(8):
                    nc.tensor.matmul(
                        psz,
                        lhsT=zwsb[:, 2 * dc:2 * dc + 2],
                        rhs=xts[dc][:, nt * 512:(nt + 1) * 512],
                        start=(dc == 0),
                        stop=(dc == 7),
                    )
                # rows [z, resid] + per-partition bias [zc0, cr]; z then fp8
                nc.vector.tensor_scalar_add(
                    residsb2[0:2, nt * 512:(nt + 1) * 512], psz[0:2, :],
                    cpack[0:2, 4:5],
                )
                nc.vector.tensor_copy(
                    out=zrow8[:, nt * 512:(nt + 1) * 512],
                    in_=residsb2[0:1, nt * 512:(nt + 1) * 512],
                )
                # ship this half to its collective input and gather it
                nc.sync.dma_start(
                    out=cc_in[nt][0:H, :].rearrange("(i p) c -> p i c", p=128),
                    in_=ktl8[:, :, nt * 512:(nt + 1) * 512],
                )
                nc.sync.dma_start(
                    out=cc_in[nt][H:H + 1, :], in_=zrow8[:, nt * 512:(nt + 1) * 512]
                )
                nc.gpsimd.collective_compute(
                    "AllGather",
                    mybir.AluOpType.bypass,
                    replica_groups=[list(range(NC))],
                    ins=[cc_in[nt][:]],
                    outs=[cc_out[nt][:]],
                )

            # resid to partition 0 for the epilogue
            residsb = consts.tile([1, S], f32)
            nc.sync.dma_start(out=residsb, in_=residsb2[1:2, :])

            # ---- qT (overlaps the collectives) ----
            qt8 = work.tile([128, 2, S], fp8)
            for hc in range(2):
                for nt in range(2):
                    ps = stp.tile([128, 512], f32, tag="st", name="ps")
                    for dc in range(8):
                        nc.tensor.matmul(
                            ps,
                            lhsT=wq1sb[:, dc, hc * 128:(hc + 1) * 128],
                            rhs=xts[dc][:, nt * 512:(nt + 1) * 512],
                            start=(dc == 0),
                            stop=(dc == 7),
                        )
                    nc.scalar.activation(
                        out=qt8[:, hc, nt * 512:(nt + 1) * 512], in_=ps,
                        func=Ident, bias=cpack[:, 2 + hc:2 + hc + 1], scale=SCQ,
                    )

            # ---- constants for transposes / reduce stationaries ----
            ident = consts.tile([16, 16], f32)
            make_identity(nc, ident)
            ident8 = consts.tile([8, 8], fp8)
            nc.vector.tensor_copy(out=ident8, in_=ident[0:8, 0:8])
            # z columns in fp8, one per (rank, chunk-in-rank): zcat8[:, r*8+c]
            zcat8 = consts.tile([128, 64], fp8)
            # all 32 pair stationaries [128, pair, 2, 128]: cols 0:64 the two
            # z chunks replicated (built per half with broadcast DVE copies),
            # cols 64:128 ones.  Pair pi = t*16 + r*2 + cp handles global
            # chunks g = r*8 + t*4 + 2*cp + j.
            zrall = consts.tile([128, 32, 2, 128], fp8)
            for j in range(2):
                nc.vector.memset(zrall[:, :, j, 64:128], 1.0)
            zr6 = zrall.rearrange("p (t r cp) j c -> p t r cp j c", t=2, cp=2)
            zc3 = zcat8.rearrange("p (r f) -> p r f", f=8)

            # ---- main loop ----
            kt8 = work.tile([128, 2, N], fp8)
            cc3 = [cc_out[t].rearrange("(r q) c -> r q c", q=H + 1) for t in range(2)]
            zrows8 = [work.tile([8, 512], fp8, name=f"zrows8_{t}") for t in range(2)]

            psred = [
                redp.tile([128, 512], f32, tag=f"red{t}", name=f"psred{t}")
                for t in range(2)
            ]

            cc3q = [
                cc_out[t].rearrange("(r q) c -> q r c", q=H + 1) for t in range(2)
            ]
            kt8r = kt8.rearrange("p i (r n) -> p i r n", r=NC)

            def emit_load_half(t):
                # kt columns: rank r half t -> global cols r*S + t*512;
                # one DMA per hc half across all ranks, one for the z rows
                for i in range(2):
                    nc.sync.dma_start(
                        out=kt8r[:, i, :, t * 512:(t + 1) * 512],
                        in_=cc3q[t][i * 128:(i + 1) * 128, :, :],
                    )
                nc.sync.dma_start(out=zrows8[t], in_=cc3[t][:, H, :])
                for f in range(4):
                    # fp8 transpose mode requires output element step of 2
                    pzt = stp.tile([128, 8, 2], fp8, tag="st", name="pzt")
                    nc.tensor.transpose(
                        out=pzt[:, :, 0], in_=zrows8[t][:, f * 128:(f + 1) * 128],
                        identity=ident8,
                    )
                    nc.vector.tensor_copy(
                        out=zc3[:, :, t * 4 + f], in_=pzt[:, :, 0]
                    )
                # replicate this half's z columns into the 16 pair
                # stationaries with two broadcast copies
                for cp in range(2):
                    src = zc3[:, :, t * 4 + 2 * cp: t * 4 + 2 * cp + 2]
                    nc.vector.tensor_copy(
                        out=zr6[:, t, :, cp, :, 0:64],
                        in_=src[:, :, :, None].broadcast_to([128, 8, 2, 64]),
                    )

            # pair schedule: 16 pairs per column half; each pair handles two
            # adjacent global 128-chunks of one rank
            sched = []
            for t in range(2):
                for r in range(NC):
                    for cp in range(2):
                        chunks = []
                        for j in range(2):
                            c = t * 4 + 2 * cp + j      # chunk-in-rank 0..7
                            g = r * 8 + c               # global chunk
                            chunks.append(kt8[:, :, g * 128:(g + 1) * 128])
                        sched.append(chunks)

            e2s = {}

            def emit_score(pi):
                chunks = sched[pi]
                e2 = expp.tile([128, 2, S], fp8, tag="e2", name="e2")
                for j, ktap in enumerate(chunks):
                    st = stp.tile([128, 1024], f32, tag="st", name="st")
                    for nt in range(2):
                        nc.tensor.matmul(
                            st[:, nt * 512:(nt + 1) * 512],
                            lhsT=ktap,
                            rhs=qt8[:, :, nt * 512:(nt + 1) * 512],
                            perf_mode=DR,
                        )
                    # ~60/40 ACT/DVE split: j==1 tiles go to the DVE
                    # (Schraudolph) except every 5th pair
                    if j == 1 and pi % 5 != 4:
                        nc.vector.tensor_scalar(
                            out=e2[:, j, :].bitcast(i8), in0=st,
                            scalar1=A8, scalar2=B8, op0=Mul, op1=Add,
                        )
                    else:
                        nc.scalar.activation(
                            out=e2[:, j, :], in_=st, func=Exp, scale=SC_EXP
                        )
                e2s[pi] = e2

            def emit_reduce(pi):
                e2 = e2s.pop(pi)
                zr = zrall[:, pi, :, :]
                for nt in range(2):
                    nc.tensor.matmul(
                        psred[nt],
                        lhsT=zr,
                        rhs=e2[:, :, nt * 512:(nt + 1) * 512],
                        perf_mode=DR,
                        start=(pi == 0),
                        stop=(pi == NPAIR - 1),
                    )

            for pi in range(NPAIR + LAG):
                if pi == 0:
                    emit_load_half(0)
                if pi == 8:
                    emit_load_half(1)
                if pi < NPAIR:
                    emit_score(pi)
                if pi >= LAG:
                    emit_reduce(pi - LAG)

            # ---- epilogue: out = resid + num/den ----
            # psred rows: 0..63 num copies, 64..127 den copies
            dsb = consts.tile([65, S], f32)
            for nt in range(2):
                nc.vector.tensor_copy(
                    out=dsb[64:65, nt * 512:(nt + 1) * 512], in_=psred[nt][64:65, :]
                )
            dall = consts.tile([1, S], f32)
            nc.sync.dma_start(out=dall, in_=dsb[64:65, :])
            # 1/den via ln+exp on the ACT (vector.reciprocal measured 7.8us)
            lnd = consts.tile([1, S], f32)
            nc.scalar.activation(out=lnd, in_=dall, func=Log)
            rden = consts.tile([1, S], f32)
            nc.scalar.activation(out=rden, in_=lnd, func=Exp, scale=-1.0)
            m = consts.tile([1, S], f32)
            for nt in range(2):
                nc.vector.tensor_mul(
                    m[:, nt * 512:(nt + 1) * 512],
                    psred[nt][0:1, :],
                    rden[:, nt * 512:(nt + 1) * 512],
                )
            outsb = consts.tile([1, S], f32)
            nc.vector.tensor_add(outsb, m, residsb)
            nc.sync.dma_start(out=out_d[:], in_=outsb)

    nc.compile()
    return nc


def _get_program():
    if "nc" not in _cache:
        _cache["nc"] = _build_program()
    return _cache["nc"]


def kernel(x, lin1_w, lin1_b, q_w, q_b, k_w, k_b, v_w, v_b, lin2_w, lin2_b):
    from concourse.bass_utils import run_bass_kernel_spmd

    x = np.asarray(x, dtype=np.float32)
    lin1_w = np.asarray(lin1_w, dtype=np.float32)
    lin1_b = np.asarray(lin1_b, dtype=np.float32)
    q_w = np.asarray(q_w, dtype=np.float32)
    q_b = np.asarray(q_b, dtype=np.float32)
    k_w = np.asarray(k_w, dtype=np.float32)
    k_b = np.asarray(k_b, dtype=np.float32)
    v_w = np.asarray(v_w, dtype=np.float32)
    v_b = np.asarray(v_b, dtype=np.float32)
    lin2_w = np.asarray(lin2_w, dtype=np.float32)
    lin2_b = np.asarray(lin2_b, dtype=np.float32)

    nc = _get_program()

    wk1 = (k_w.astype(np.float64) @ lin1_w.astype(np.float64)).astype(np.float32)
    bk1 = (k_w.astype(np.float64) @ lin1_b.astype(np.float64)).astype(np.float32) + k_b
    wq1 = (q_w.astype(np.float64) @ lin1_w.astype(np.float64)).astype(np.float32)
    bq1 = (q_w.astype(np.float64) @ lin1_b.astype(np.float64)).astype(np.float32) + q_b
    w2 = lin2_w[0]                                  # [H]
    wv2 = (v_w.T.astype(np.float64) @ w2.astype(np.float64)).astype(np.float32)
    zw = (lin1_w.T.astype(np.float64) @ wv2.astype(np.float64)).astype(np.float32)
    zc0 = np.float32(wv2.astype(np.float64) @ lin1_b.astype(np.float64))
    wr = (lin1_w.T.astype(np.float64) @ w2.astype(np.float64)).astype(np.float32)
    cr = np.float32(lin1_b @ w2 + v_b @ w2 + lin2_b[0])

    cpk = np.zeros((128, 16), dtype=np.float32)
    cpk[:, 0:2] = (bk1 * SCQ).reshape(2, 128).T
    cpk[:, 2:4] = (bq1 * SCQ).reshape(2, 128).T
    cpk[0, 4] = zc0
    cpk[1, 4] = cr
    zwp = np.zeros((128, 16), dtype=np.float32)
    zwp[:, 0::2] = zw.reshape(8, 128).T
    zwp[:, 1::2] = wr.reshape(8, 128).T

    wk1T = np.ascontiguousarray(wk1.T)              # [D, H]
    wq1T = np.ascontiguousarray(wq1.T)              # [D, H]

    in_maps = []
    for i in range(NC):
        in_maps.append({
            "xT": np.ascontiguousarray(x[i * S:(i + 1) * S, :].T),
            "wk1T": wk1T, "wq1T": wq1T,
            "cpk": cpk, "zwp": zwp,
        })

    res = run_bass_kernel_spmd(nc, in_maps, core_ids=list(range(NC)))
    out = np.concatenate([res.results[i]["out"].reshape(S) for i in range(NC)])
    return out.astype(np.float32)
